# revision 1
# baseline (speedup 1.0000x reference)
"""Trainium2 Bass kernel for nn_CombinedModel (NMS detection + ROI classifier).

Sharding: pooled-pixel-row sharding. Core c computes conv output rows
y in [8c, 8c+8) (= pooled rows py in [4c,4c+4)) of ALL 300 ROIs, which is
exactly the k-slice S_c = {(oc, py, px): py in [4c,4c+4)} of the 16384-wide
W1 contraction. Each core runs the 8-head GEMM against its 2048-row W1
slice, a ReduceScatter sums the partial [8,128,304] and hands head c to
core c, which applies bias/relu + its head matmul + keep mask.
NMS / top-k / ROI selection is tiny and done host-side during input prep.
"""
import numpy as np

N_CORES = 8
R = 304            # 300 rois padded to 8*38
IMG = 640
INP = 64
CONF = 0.25
IOU = 0.45
K = 300
PROV, ALPHA, AD = 38, 25, 35
OUTW = 40          # padded per-core head width

_CACHE = {}


def _build_bass():
    import concourse.bacc as bacc
    import concourse.mybir as mybir
    import concourse.tile as tile

    nc = bacc.Bacc("TRN2", target_bir_lowering=False, debug=False,
                   num_devices=N_CORES)
    f32 = mybir.dt.float32
    cols = nc.dram_tensor("cols", [108, 38912], f32, kind="ExternalInput").ap()
    wstk = nc.dram_tensor("wstk", [108, 64], f32, kind="ExternalInput").ap()
    bc64 = nc.dram_tensor("bc64", [64, 1], f32, kind="ExternalInput").ap()
    w1s = nc.dram_tensor("w1s", [8, 16, 128, 128], f32, kind="ExternalInput").ap()
    b1c = nc.dram_tensor("b1c", [128, 1], f32, kind="ExternalInput").ap()
    w2 = nc.dram_tensor("w2", [128, OUTW], f32, kind="ExternalInput").ap()
    b2 = nc.dram_tensor("b2", [OUTW, 1], f32, kind="ExternalInput").ap()
    keepm = nc.dram_tensor("keepm", [OUTW, R], f32, kind="ExternalInput").ap()
    out = nc.dram_tensor("out", [OUTW, R], f32, kind="ExternalOutput").ap()

    NM = 76  # conv matmuls of 512 cols each

    with tile.TileContext(nc) as tc:
        with (
            tc.tile_pool(name="const", bufs=1) as cpool,
            tc.tile_pool(name="colsp", bufs=3) as colsp,
            tc.tile_pool(name="psum", bufs=1, space="PSUM") as psum,
            tc.tile_pool(name="work", bufs=2) as work,
            tc.tile_pool(name="dram", bufs=1, space="DRAM") as dpool,
        ):
            wstk_sb = cpool.tile([108, 64], f32)
            nc.sync.dma_start(wstk_sb[:], wstk[:])
            bc64_sb = cpool.tile([64, 1], f32)
            nc.sync.dma_start(bc64_sb[:], bc64[:])
            b1c_sb = cpool.tile([128, 1], f32)
            nc.sync.dma_start(b1c_sb[:], b1c[:])
            w2_sb = cpool.tile([128, OUTW], f32)
            nc.sync.dma_start(w2_sb[:], w2[:])
            b2_sb = cpool.tile([OUTW, 1], f32)
            nc.sync.dma_start(b2_sb[:], b2[:])
            keep_sb = cpool.tile([OUTW, R], f32)
            nc.sync.dma_start(keep_sb[:], keepm[:])

            pooled2 = cpool.tile([128, 16, R], f32)

            # conv + pool: 4 col chunks of 19 matmuls each
            CH = 19
            for ch in range(4):
                ctile = colsp.tile([108, CH * 512], f32, tag="cols", bufs=2)
                nc.sync.dma_start(ctile[:], cols[:, ch * CH * 512:(ch + 1) * CH * 512])
                for j in range(CH):
                    m = ch * CH + j
                    ps = psum.tile([64, 4, 2, 64], f32, tag="cv", bufs=4)
                    nc.tensor.matmul(ps.rearrange("p a b c -> p (a b c)"),
                                     wstk_sb[:], ctile[:, j * 512:(j + 1) * 512],
                                     start=True, stop=True)
                    craw = work.tile([64, 4, 2, 64], f32, tag="craw")
                    nc.scalar.activation(
                        craw.rearrange("p a b c -> p (a b c)"),
                        ps.rearrange("p a b c -> p (a b c)"),
                        mybir.ActivationFunctionType.Relu,
                        bias=bc64_sb[:])
                    t0 = work.tile([64, 4, 32], f32, tag="t0")
                    t1 = work.tile([64, 4, 32], f32, tag="t1")
                    nc.vector.tensor_tensor(out=t0[:], in0=craw[:, :, 0, 0::2],
                                            in1=craw[:, :, 0, 1::2],
                                            op=mybir.AluOpType.max)
                    nc.vector.tensor_tensor(out=t1[:], in0=craw[:, :, 1, 0::2],
                                            in1=craw[:, :, 1, 1::2],
                                            op=mybir.AluOpType.max)
                    nc.vector.tensor_tensor(
                        out=pooled2[0:64, :, 4 * m:4 * m + 4].rearrange(
                            "p x r -> p r x"),
                        in0=t0[:, :, 0::2], in1=t1[:, :, 0::2],
                        op=mybir.AluOpType.max)
                    nc.vector.tensor_tensor(
                        out=pooled2[64:128, :, 4 * m:4 * m + 4].rearrange(
                            "p x r -> p r x"),
                        in0=t0[:, :, 1::2], in1=t1[:, :, 1::2],
                        op=mybir.AluOpType.max)

            # 8-head GEMM over this core's 2048-row W1 slice
            import os
            STAGE = int(os.environ.get("KSTAGE", "3"))

            if STAGE == 0:
                om0 = work.tile([OUTW, R], f32, tag="om")
                nc.vector.tensor_copy(om0[:], pooled2[:OUTW, 0, :])
                nc.sync.dma_start(out[:], om0[:])
            if STAGE >= 1:
                parts = cpool.tile([128, 8, R], f32)
                for h in range(8):
                    w1h = colsp.tile([128, 16, 128], f32, tag="w1h", bufs=2)
                    nc.sync.dma_start(w1h[:], w1s[h].rearrange("q k d -> k q d"))
                    ph = psum.tile([128, R], f32, tag="gemm", bufs=2)
                    for q in range(16):
                        nc.tensor.matmul(ph[:], w1h[:, q, :], pooled2[:, q, :],
                                         start=(q == 0), stop=(q == 15))
                    nc.vector.tensor_copy(parts[:, h, :], ph[:])
            if STAGE == 1:
                om1 = work.tile([OUTW, R], f32, tag="om")
                nc.vector.tensor_copy(om1[:], parts[:OUTW, 0, :])
                nc.sync.dma_start(out[:], om1[:])
            if STAGE >= 2:
                cc_in = dpool.tile([8, 128, R], f32)
                cc_out = dpool.tile([128, R], f32)
                nc.sync.dma_start(cc_in.rearrange("h p r -> p h r"), parts[:])
                nc.gpsimd.collective_compute(
                    "ReduceScatter", mybir.AluOpType.add,
                    ins=[cc_in[:]], outs=[cc_out[:]],
                    replica_groups=[list(range(N_CORES))],
                )
                hsb = work.tile([128, R], f32, tag="hsb")
                nc.sync.dma_start(hsb[:], cc_out[:])
                hrelu = work.tile([128, R], f32, tag="hrelu")
                nc.scalar.activation(hrelu[:], hsb[:],
                                     mybir.ActivationFunctionType.Relu,
                                     bias=b1c_sb[:])
                po = psum.tile([OUTW, R], f32, tag="head")
                nc.tensor.matmul(po[:], w2_sb[:], hrelu[:], start=True, stop=True)
                ob = work.tile([OUTW, R], f32, tag="ob")
                nc.vector.tensor_scalar(ob[:], po[:], b2_sb[:], None,
                                        op0=mybir.AluOpType.add)
                om = work.tile([OUTW, R], f32, tag="om")
                nc.vector.tensor_tensor(out=om[:], in0=ob[:], in1=keep_sb[:],
                                        op=mybir.AluOpType.mult)
                nc.sync.dma_start(out[:], om[:])
    nc.compile()
    return nc


def _host_prep(preds, image, W_conv, b_conv, W1, b1, W2p, b2p, W2a, b2a, W2d, b2d):
    p = preds[0].astype(np.float32)
    score = p[:, 4] * p[:, 5]
    masked = np.where(score > CONF, score, -np.inf)
    idx = np.argsort(-masked, kind="stable")[:K]
    top_s = masked[idx]
    xy, wh = p[:, 0:2], p[:, 2:4]
    boxes = np.concatenate([xy - wh / 2, xy + wh / 2], axis=-1)
    b = boxes[idx]
    valid = top_s > CONF
    x1, y1, x2, y2 = b[:, 0], b[:, 1], b[:, 2], b[:, 3]
    area = (x2 - x1) * (y2 - y1)
    iw = np.clip(np.minimum(x2[:, None], x2[None, :]) - np.maximum(x1[:, None], x1[None, :]), 0, None)
    ih = np.clip(np.minimum(y2[:, None], y2[None, :]) - np.maximum(y1[:, None], y1[None, :]), 0, None)
    iou = iw * ih / (area[:, None] + area[None, :] - iw * ih + 1e-7)
    keep = valid.copy()
    for i in range(K):
        sup = np.any((iou[i, :i] > IOU) & keep[:i])
        keep[i] = keep[i] & ~sup

    xi = np.clip(np.round(b[:, 0]).astype(np.int32), 0, IMG - INP)
    yi = np.clip(np.round(b[:, 1]).astype(np.int32), 0, IMG - INP)
    img0 = image[0]
    pad = np.zeros((R, 3, 66, 66), np.float32)
    for r in range(K):
        pad[r, :, 1:65, 1:65] = img0[:, yi[r]:yi[r] + 64, xi[r]:xi[r] + 64]

    from numpy.lib.stride_tricks import sliding_window_view
    # patches[roi, c, yy, x, ky, kx]
    patches = sliding_window_view(pad, (3, 3), axis=(2, 3))
    P2 = np.ascontiguousarray(patches.transpose(2, 1, 4, 5, 0, 3))  # [yy,c,ky,kx,roi,x]
    cols_all = np.ascontiguousarray(
        P2.reshape(8, 4, 2, 27, R, 64).transpose(0, 1, 3, 4, 2, 5)
    ).reshape(8, 108, 38912)

    wstk = np.zeros((108, 64), np.float32)
    wc = W_conv.reshape(16, 27).T  # [27, 16]
    for ph in range(4):
        wstk[ph * 27:(ph + 1) * 27, ph * 16:(ph + 1) * 16] = wc
    bc64 = np.tile(b_conv.astype(np.float32), 4).reshape(64, 1)

    # w1s[core][h, px, py*16+oc, d]
    W1r = W1.reshape(8, 16, 32, 32, 128)  # [h, oc, py, px, d]
    w1s_all = np.empty((8, 8, 16, 128, 128), np.float32)
    for core in range(8):
        blk = W1r[:, :, 4 * core:4 * core + 4, :, :]       # [h, oc, py4, px, d]
        t = np.ascontiguousarray(blk.transpose(0, 3, 2, 1, 4))  # [h, px, py, oc, d]
        w1s_all[core] = t.reshape(8, 16, 2, 64, 128).reshape(8, 16, 128, 128)

    w2_all = np.zeros((8, 128, OUTW), np.float32)
    b2_all = np.zeros((8, OUTW, 1), np.float32)
    w2_all[0, :, :PROV] = W2p; b2_all[0, :PROV, 0] = b2p
    w2_all[1, :, :ALPHA] = W2a; b2_all[1, :ALPHA, 0] = b2a
    for j in range(6):
        w2_all[2 + j, :, :AD] = W2d[j]; b2_all[2 + j, :AD, 0] = b2d[j]

    keepf = np.zeros((R,), np.float32)
    keepf[:K] = keep.astype(np.float32)
    keepm = np.broadcast_to(keepf, (OUTW, R)).copy()

    in_maps = []
    for core in range(8):
        in_maps.append({
            "cols": cols_all[core],
            "wstk": wstk,
            "bc64": bc64,
            "w1s": w1s_all[core],
            "b1c": b1[core].reshape(128, 1).astype(np.float32),
            "w2": w2_all[core],
            "b2": b2_all[core],
            "keepm": keepm,
        })
    return in_maps


def kernel(**inputs):
    from concourse import bass_utils
    if "nc" not in _CACHE:
        _CACHE["nc"] = _build_bass()
    nc = _CACHE["nc"]
    in_maps = _host_prep(**{k: np.asarray(v) for k, v in inputs.items()})
    res = bass_utils.run_bass_kernel_spmd(nc, in_maps, core_ids=list(range(N_CORES)))
    _CACHE["last_res"] = res
    outs = [res.results[c]["out"].T for c in range(N_CORES)]  # [304, 40] each
    logits = np.concatenate(
        [outs[0][:K, :PROV], outs[1][:K, :ALPHA]]
        + [outs[2 + j][:K, :AD] for j in range(6)], axis=1)
    return logits.astype(np.float32)



# revision 3
# speedup vs baseline: 1.3386x; 1.3386x over previous
"""Trainium2 Bass kernel for nn_CombinedModel (NMS detection + ROI classifier).

Sharding: pooled-pixel-row sharding. Core c computes conv output rows
y in [8c, 8c+8) (= pooled rows py in [4c,4c+4)) of ALL 300 ROIs, which is
exactly the k-slice of the 16384-wide W1 contraction. Each core runs the
8-head GEMM against its 2048-row W1 slice, a ReduceScatter sums the
partial [8,128,304] and hands head c to core c, which applies bias/relu +
its head matmul + keep mask. NMS / top-k / ROI selection is tiny and done
host-side during input prep.

v2: bf16 datapath (fp32 PSUM accumulate), col-paired conv matmuls for
128-partition ACT/DVE, ReLU via ACT then 2-stage contiguous max-pool on
DVE, W1 fully resident in SBUF, bf16 ReduceScatter.
"""
import numpy as np

N_CORES = 8
R = 304            # 300 rois padded to 8*38
IMG = 640
INP = 64
CONF = 0.25
IOU = 0.45
K = 300
PROV, ALPHA, AD = 38, 25, 35
OUTW = 40          # padded per-core head width
NT = 38            # conv pair-tiles (8 rois each)

_CACHE = {}


def _build_bass():
    import concourse.bacc as bacc
    import concourse.mybir as mybir
    import concourse.tile as tile

    nc = bacc.Bacc("TRN2", target_bir_lowering=False, debug=False,
                   num_devices=N_CORES)
    f32 = mybir.dt.float32
    bf16 = mybir.dt.bfloat16
    cols = nc.dram_tensor("cols", [108, NT * 1024], bf16, kind="ExternalInput").ap()
    wstk = nc.dram_tensor("wstk", [108, 64], bf16, kind="ExternalInput").ap()
    bc128 = nc.dram_tensor("bc128", [128, 1], f32, kind="ExternalInput").ap()
    w1s = nc.dram_tensor("w1s", [128, 8, 16, 128], bf16, kind="ExternalInput").ap()
    b1c = nc.dram_tensor("b1c", [128, 1], f32, kind="ExternalInput").ap()
    w2 = nc.dram_tensor("w2", [128, OUTW], bf16, kind="ExternalInput").ap()
    b2 = nc.dram_tensor("b2", [OUTW, 1], f32, kind="ExternalInput").ap()
    keepm = nc.dram_tensor("keepm", [OUTW, R], f32, kind="ExternalInput").ap()
    out = nc.dram_tensor("out", [OUTW, R], f32, kind="ExternalOutput").ap()

    with tile.TileContext(nc) as tc:
        with (
            tc.tile_pool(name="const", bufs=1) as cpool,
            tc.tile_pool(name="colsp", bufs=2) as colsp,
            tc.tile_pool(name="psum", bufs=1, space="PSUM") as psum,
            tc.tile_pool(name="work", bufs=2) as work,
            tc.tile_pool(name="dram", bufs=1, space="DRAM") as dpool,
        ):
            wstk_sb = cpool.tile([108, 64], bf16)
            nc.sync.dma_start(wstk_sb[:], wstk[:])
            bc128_sb = cpool.tile([128, 1], f32)
            nc.sync.dma_start(bc128_sb[:], bc128[:])
            b1c_sb = cpool.tile([128, 1], f32)
            nc.sync.dma_start(b1c_sb[:], b1c[:])
            w2_sb = cpool.tile([128, OUTW], bf16)
            nc.sync.dma_start(w2_sb[:], w2[:])
            b2_sb = cpool.tile([OUTW, 1], f32)
            nc.sync.dma_start(b2_sb[:], b2[:])
            keep_sb = cpool.tile([OUTW, R], f32)
            nc.sync.dma_start(keep_sb[:], keepm[:])

            # all of this core's W1 slice stays resident (32KB/partition)
            w1all = cpool.tile([128, 8, 16, 128], bf16)
            nc.sync.dma_start(w1all[:], w1s[:])

            # pooled features: col = tile*128 + grp*64 + roi*16 + q
            pooled2 = cpool.tile([128, NT, 2, 4, 16], bf16)

            # ---- conv + relu + maxpool, 4 pair-tiles per cols chunk ----
            CB = 4
            for B in range(0, NT, CB):
                nb = min(CB, NT - B)
                ctile = colsp.tile([108, nb * 1024], bf16, tag="cols", bufs=2)
                nc.sync.dma_start(ctile[:], cols[:, B * 1024:(B + nb) * 1024])
                tyb = work.tile([128, CB, 4, 64], bf16, tag="ty", bufs=2)
                for j in range(nb):
                    ps = psum.tile([128, 512], f32, tag="cv", bufs=4)
                    nc.tensor.matmul(ps[0:64, :], wstk_sb[:],
                                     ctile[:, j * 1024:j * 1024 + 512],
                                     start=True, stop=True)
                    nc.tensor.matmul(ps[64:128, :], wstk_sb[:],
                                     ctile[:, j * 1024 + 512:(j + 1) * 1024],
                                     start=True, stop=True)
                    craw = work.tile([128, 4, 2, 64], bf16, tag="craw", bufs=3)
                    nc.scalar.activation(
                        craw.rearrange("p a b c -> p (a b c)"), ps[:],
                        mybir.ActivationFunctionType.Relu, bias=bc128_sb[:])
                    # pool over ydup (y pairs)
                    nc.vector.tensor_tensor(out=tyb[:, j, :, :],
                                            in0=craw[:, :, 0, :],
                                            in1=craw[:, :, 1, :],
                                            op=mybir.AluOpType.max)
                # pool over par (x pairs), split px halves across partitions
                for g in range(2):
                    for pxh in range(2):
                        nc.vector.tensor_tensor(
                            out=pooled2[64 * pxh:64 * pxh + 64, B:B + nb, g, :, :],
                            in0=tyb[64 * g:64 * g + 64, 0:nb, :,
                                    pxh * 16:pxh * 16 + 16],
                            in1=tyb[64 * g:64 * g + 64, 0:nb, :,
                                    32 + pxh * 16:32 + pxh * 16 + 16],
                            op=mybir.AluOpType.max)

            # ---- 8-head GEMM over this core's 2048-row W1 slice ----
            parts = cpool.tile([128, 8, R], bf16)
            cc_in = dpool.tile([8, 128, R], bf16)
            cc_out = dpool.tile([128, R], bf16)
            for h in range(8):
                ph_ps = psum.tile([128, R], f32, tag="gemm", bufs=2)
                for q in range(16):
                    nc.tensor.matmul(ph_ps[:], w1all[:, h, q, :],
                                     pooled2[:, :, :, :, q],
                                     start=(q == 0), stop=(q == 15))
                nc.scalar.activation(parts[:, h, :], ph_ps[:],
                                     mybir.ActivationFunctionType.Copy)
                nc.sync.dma_start(cc_in[h], parts[:, h, :])

            nc.gpsimd.collective_compute(
                "ReduceScatter", mybir.AluOpType.add,
                ins=[cc_in[:]], outs=[cc_out[:]],
                replica_groups=[list(range(N_CORES))],
            )
            hsb = work.tile([128, R], bf16, tag="hsb")
            nc.sync.dma_start(hsb[:], cc_out[:])
            hrelu = work.tile([128, R], bf16, tag="hrelu")
            nc.scalar.activation(hrelu[:], hsb[:],
                                 mybir.ActivationFunctionType.Relu,
                                 bias=b1c_sb[:])
            po = psum.tile([OUTW, R], f32, tag="head")
            nc.tensor.matmul(po[:], w2_sb[:], hrelu[:], start=True, stop=True)
            ob = work.tile([OUTW, R], f32, tag="ob")
            nc.vector.tensor_scalar(ob[:], po[:], b2_sb[:], None,
                                    op0=mybir.AluOpType.add)
            om = work.tile([OUTW, R], f32, tag="om")
            nc.vector.tensor_tensor(out=om[:], in0=ob[:], in1=keep_sb[:],
                                    op=mybir.AluOpType.mult)
            nc.sync.dma_start(out[:], om[:])
    nc.compile()
    return nc


def _host_prep(preds, image, W_conv, b_conv, W1, b1, W2p, b2p, W2a, b2a, W2d, b2d):
    import ml_dtypes
    bf16 = ml_dtypes.bfloat16
    p = preds[0].astype(np.float32)
    score = p[:, 4] * p[:, 5]
    masked = np.where(score > CONF, score, -np.inf)
    idx = np.argsort(-masked, kind="stable")[:K]
    top_s = masked[idx]
    xy, wh = p[:, 0:2], p[:, 2:4]
    boxes = np.concatenate([xy - wh / 2, xy + wh / 2], axis=-1)
    b = boxes[idx]
    valid = top_s > CONF
    x1, y1, x2, y2 = b[:, 0], b[:, 1], b[:, 2], b[:, 3]
    area = (x2 - x1) * (y2 - y1)
    iw = np.clip(np.minimum(x2[:, None], x2[None, :]) - np.maximum(x1[:, None], x1[None, :]), 0, None)
    ih = np.clip(np.minimum(y2[:, None], y2[None, :]) - np.maximum(y1[:, None], y1[None, :]), 0, None)
    iou = iw * ih / (area[:, None] + area[None, :] - iw * ih + 1e-7)
    keep = valid.copy()
    for i in range(K):
        sup = np.any((iou[i, :i] > IOU) & keep[:i])
        keep[i] = keep[i] & ~sup

    xi = np.clip(np.round(b[:, 0]).astype(np.int32), 0, IMG - INP)
    yi = np.clip(np.round(b[:, 1]).astype(np.int32), 0, IMG - INP)
    img0 = image[0]
    pad = np.zeros((R, 3, 66, 66), np.float32)
    for r in range(K):
        pad[r, :, 1:65, 1:65] = img0[:, yi[r]:yi[r] + 64, xi[r]:xi[r] + 64]

    from numpy.lib.stride_tricks import sliding_window_view
    # patches[roi, ic, y, x, ky, kx]
    patches = sliding_window_view(pad, (3, 3), axis=(2, 3))
    cols_all = np.empty((8, 108, NT * 1024), bf16)
    for c in range(8):
        sub = patches[:, :, 8 * c:8 * c + 8]            # [R,3,8,64,3,3]
        arr = sub.reshape(R, 3, 4, 2, 2, 16, 2, 3, 3)   # roi,ic,ph,ydup,pxh,px,par,ky,kx
        cols_all[c] = arr.transpose(2, 1, 7, 8, 0, 3, 6, 4, 5).reshape(108, NT * 1024)

    wstk = np.zeros((108, 64), np.float32)
    wc = W_conv.reshape(16, 27).T  # [27, 16]
    for ph in range(4):
        wstk[ph * 27:(ph + 1) * 27, ph * 16:(ph + 1) * 16] = wc
    bc128 = np.tile(b_conv.astype(np.float32), 8).reshape(128, 1)

    # w1s[core][k=(pxh,ph,ch), h, q=px%16, d]
    W1r = W1.reshape(8, 16, 32, 32, 128)  # [h, ch, py, px, d]
    w1s_all = np.empty((8, 128, 8, 16, 128), bf16)
    for c in range(8):
        blk = W1r[:, :, 4 * c:4 * c + 4, :, :]          # [h, ch, ph, px32, d]
        t = blk.reshape(8, 16, 4, 2, 16, 128)           # [h,ch,ph,pxh,px,d]
        w1s_all[c] = t.transpose(3, 2, 1, 0, 4, 5).reshape(128, 8, 16, 128)

    w2_all = np.zeros((8, 128, OUTW), np.float32)
    b2_all = np.zeros((8, OUTW, 1), np.float32)
    w2_all[0, :, :PROV] = W2p; b2_all[0, :PROV, 0] = b2p
    w2_all[1, :, :ALPHA] = W2a; b2_all[1, :ALPHA, 0] = b2a
    for j in range(6):
        w2_all[2 + j, :, :AD] = W2d[j]; b2_all[2 + j, :AD, 0] = b2d[j]

    keepf = np.zeros((R,), np.float32)
    keepf[:K] = keep.astype(np.float32)
    keepm = np.broadcast_to(keepf, (OUTW, R)).copy()

    in_maps = []
    for c in range(8):
        in_maps.append({
            "cols": cols_all[c],
            "wstk": wstk.astype(bf16),
            "bc128": bc128,
            "w1s": w1s_all[c],
            "b1c": b1[c].reshape(128, 1).astype(np.float32),
            "w2": w2_all[c].astype(bf16),
            "b2": b2_all[c],
            "keepm": keepm,
        })
    return in_maps


def kernel(**inputs):
    from concourse import bass_utils
    if "nc" not in _CACHE:
        _CACHE["nc"] = _build_bass()
    nc = _CACHE["nc"]
    in_maps = _host_prep(**{k: np.asarray(v) for k, v in inputs.items()})
    res = bass_utils.run_bass_kernel_spmd(nc, in_maps, core_ids=list(range(N_CORES)))
    _CACHE["last_res"] = res
    outs = [res.results[c]["out"].T for c in range(N_CORES)]  # [304, 40] each
    logits = np.concatenate(
        [outs[0][:K, :PROV], outs[1][:K, :ALPHA]]
        + [outs[2 + j][:K, :AD] for j in range(6)], axis=1)
    return logits.astype(np.float32)


# revision 8
# speedup vs baseline: 1.7689x; 1.3215x over previous
"""Trainium2 Bass kernel for nn_CombinedModel (NMS detection + ROI classifier).

Sharding: pooled-pixel-row sharding. Core c computes conv output rows
y in [8c, 8c+8) (= pooled rows py in [4c,4c+4)) of ALL 300 ROIs, which is
exactly the k-slice of the 16384-wide W1 contraction. Each core runs the
8-head GEMM against its 2048-row W1 slice, a ReduceScatter sums the
partial [8,128,304] and hands head c to core c, which applies bias/relu +
its head matmul + keep mask. NMS / top-k / ROI selection is tiny and done
host-side during input prep.

v2: bf16 datapath (fp32 PSUM accumulate), col-paired conv matmuls for
128-partition ACT/DVE, ReLU via ACT then 2-stage contiguous max-pool on
DVE, W1 fully resident in SBUF, bf16 ReduceScatter.
"""
import numpy as np

N_CORES = 8
R = 304            # 300 rois padded to 8*38
IMG = 640
INP = 64
CONF = 0.25
IOU = 0.45
K = 300
PROV, ALPHA, AD = 38, 25, 35
OUTW = 40          # padded per-core head width
NT = 38            # conv pair-tiles (8 rois each)

_CACHE = {}


def _build_bass():
    import os
    OPT_REARR = os.environ.get("KOPT_REARR", "1") == "1"
    OPT_GPS = os.environ.get("KOPT_GPS", "0") == "1"
    OPT_A2A = os.environ.get("KOPT_A2A", "1") == "1"
    import concourse.bacc as bacc
    import concourse.mybir as mybir
    import concourse.tile as tile

    nc = bacc.Bacc("TRN2", target_bir_lowering=False, debug=False,
                   num_devices=N_CORES)
    f32 = mybir.dt.float32
    bf16 = mybir.dt.bfloat16
    cols = nc.dram_tensor("cols", [108, NT * 1024], bf16, kind="ExternalInput").ap()
    wstk = nc.dram_tensor("wstk", [108, 64], bf16, kind="ExternalInput").ap()
    bc128 = nc.dram_tensor("bc128", [128, 1], f32, kind="ExternalInput").ap()
    w1s = nc.dram_tensor("w1s", [128, 8, 16, 128], bf16, kind="ExternalInput").ap()
    b1c = nc.dram_tensor("b1c", [128, 1], f32, kind="ExternalInput").ap()
    w2 = nc.dram_tensor("w2", [128, OUTW], bf16, kind="ExternalInput").ap()
    b2 = nc.dram_tensor("b2", [OUTW, 1], f32, kind="ExternalInput").ap()
    keepm = nc.dram_tensor("keepm", [OUTW, R], f32, kind="ExternalInput").ap()
    out = nc.dram_tensor("out", [OUTW, R], f32, kind="ExternalOutput").ap()

    with tile.TileContext(nc) as tc:
        with (
            tc.tile_pool(name="const", bufs=1) as cpool,
            tc.tile_pool(name="colsp", bufs=2) as colsp,
            tc.tile_pool(name="psum", bufs=1, space="PSUM") as psum,
            tc.tile_pool(name="work", bufs=2) as work,
            tc.tile_pool(name="dram", bufs=1, space="DRAM") as dpool,
        ):
            wstk_sb = cpool.tile([108, 64], bf16)
            nc.sync.dma_start(wstk_sb[:], wstk[:])
            bc128_sb = cpool.tile([128, 1], f32)
            nc.sync.dma_start(bc128_sb[:], bc128[:])
            b1c_sb = cpool.tile([128, 1], f32)
            nc.sync.dma_start(b1c_sb[:], b1c[:])
            w2_sb = cpool.tile([128, OUTW], bf16)
            nc.sync.dma_start(w2_sb[:], w2[:])
            b2_sb = cpool.tile([OUTW, 1], f32)
            nc.sync.dma_start(b2_sb[:], b2[:])
            keep_sb = cpool.tile([OUTW, R], f32)
            nc.sync.dma_start(keep_sb[:], keepm[:])

            # all of this core's W1 slice stays resident (32KB/partition)
            w1all = cpool.tile([128, 8, 16, 128], bf16)

            # pooled features, q-major: col = q*304 + tile*8 + grp*4 + roi
            if OPT_REARR:
                pooled2 = cpool.tile([128, 16, NT, 2, 4], bf16)
            else:
                pooled2 = cpool.tile([128, NT, 2, 4, 16], bf16)

            # ---- conv + relu + maxpool, 4 pair-tiles per cols chunk ----
            CB = 4
            for B in range(0, NT, CB):
                nb = min(CB, NT - B)
                bi = B // CB
                ctile = colsp.tile([108, nb * 1024], bf16, tag="cols", bufs=2)
                nc.sync.dma_start(ctile[:], cols[:, B * 1024:(B + nb) * 1024])
                if bi < 4:  # stream W1 in quarters behind the cols chunks
                    nc.sync.dma_start(w1all[:, 2 * bi:2 * bi + 2],
                                      w1s[:, 2 * bi:2 * bi + 2])
                tyb = work.tile([128, CB, 4, 64], bf16, tag="ty", bufs=2)
                for j in range(nb):
                    ps = psum.tile([128, 512], f32, tag="cv", bufs=4)
                    nc.tensor.matmul(ps[0:64, :], wstk_sb[:],
                                     ctile[:, j * 1024:j * 1024 + 512],
                                     start=True, stop=True)
                    nc.tensor.matmul(ps[64:128, :], wstk_sb[:],
                                     ctile[:, j * 1024 + 512:(j + 1) * 1024],
                                     start=True, stop=True)
                    craw = work.tile([128, 4, 2, 64], bf16, tag="craw", bufs=3)
                    if (j % 2 == 0) or not OPT_GPS:
                        nc.scalar.activation(
                            craw.rearrange("p a b c -> p (a b c)"), ps[:],
                            mybir.ActivationFunctionType.Relu, bias=bc128_sb[:])
                    else:
                        nc.gpsimd.tensor_scalar(
                            craw.rearrange("p a b c -> p (a b c)"), ps[:],
                            bc128_sb[:], 0.0,
                            op0=mybir.AluOpType.add, op1=mybir.AluOpType.max)
                    # pool over ydup (y pairs)
                    nc.vector.tensor_tensor(out=tyb[:, j, :, :],
                                            in0=craw[:, :, 0, :],
                                            in1=craw[:, :, 1, :],
                                            op=mybir.AluOpType.max)
                # pool over par (x pairs), split px halves across partitions
                for g in range(2):
                    for pxh in range(2):
                        if OPT_REARR:
                            o = pooled2[64 * pxh:64 * pxh + 64, :, B:B + nb,
                                        g, :].rearrange("p q t r -> p t r q")
                        else:
                            o = pooled2[64 * pxh:64 * pxh + 64, B:B + nb, g, :, :]
                        nc.vector.tensor_tensor(
                            out=o,
                            in0=tyb[64 * g:64 * g + 64, 0:nb, :,
                                    pxh * 16:pxh * 16 + 16],
                            in1=tyb[64 * g:64 * g + 64, 0:nb, :,
                                    32 + pxh * 16:32 + pxh * 16 + 16],
                            op=mybir.AluOpType.max)

            # ---- 8-head GEMM over this core's 2048-row W1 slice ----
            parts = cpool.tile([128, 8, R], bf16)
            cc_in = dpool.tile([8, 128, R], bf16)
            cc_out = dpool.tile([8, 128, R], bf16)
            for h in range(8):
                ph_ps = psum.tile([128, R], f32, tag="gemm", bufs=2)
                for q in range(16):
                    rhs = pooled2[:, q] if OPT_REARR else pooled2[:, :, :, :, q]
                    nc.tensor.matmul(ph_ps[:], w1all[:, h, q, :],
                                     rhs, start=(q == 0),
                                     stop=(q == 15))
                nc.scalar.activation(parts[:, h, :], ph_ps[:],
                                     mybir.ActivationFunctionType.Copy)
                nc.sync.dma_start(cc_in[h], parts[:, h, :])

            if OPT_A2A:
                # exchange partials (one permute step), reduce locally
                nc.gpsimd.collective_compute(
                    "AllToAll", mybir.AluOpType.bypass,
                    ins=[cc_in[:]], outs=[cc_out[:]],
                    replica_groups=[list(range(N_CORES))],
                )
                hsb8 = work.tile([128, 8, R], bf16, tag="hsb8")
                nc.sync.dma_start(hsb8[:], cc_out.rearrange("h p r -> p h r"))
                s4 = work.tile([128, 4, R], bf16, tag="s4")
                nc.vector.tensor_tensor(out=s4[:], in0=hsb8[:, 0:4, :],
                                        in1=hsb8[:, 4:8, :],
                                        op=mybir.AluOpType.add)
                s2 = work.tile([128, 2, R], bf16, tag="s2")
                nc.vector.tensor_tensor(out=s2[:], in0=s4[:, 0:2, :],
                                        in1=s4[:, 2:4, :],
                                        op=mybir.AluOpType.add)
                s1 = work.tile([128, R], bf16, tag="s1")
                nc.vector.tensor_tensor(out=s1[:], in0=s2[:, 0, :],
                                        in1=s2[:, 1, :],
                                        op=mybir.AluOpType.add)
            else:
                cc_out1 = dpool.tile([128, R], bf16)
                nc.gpsimd.collective_compute(
                    "ReduceScatter", mybir.AluOpType.add,
                    ins=[cc_in[:]], outs=[cc_out1[:]],
                    replica_groups=[list(range(N_CORES))],
                )
                s1 = work.tile([128, R], bf16, tag="s1")
                nc.sync.dma_start(s1[:], cc_out1[:])
            hrelu = work.tile([128, R], bf16, tag="hrelu")
            nc.scalar.activation(hrelu[:], s1[:],
                                 mybir.ActivationFunctionType.Relu,
                                 bias=b1c_sb[:])
            po = psum.tile([OUTW, R], f32, tag="head")
            nc.tensor.matmul(po[:], w2_sb[:], hrelu[:], start=True, stop=True)
            ob = work.tile([OUTW, R], f32, tag="ob")
            nc.vector.tensor_scalar(ob[:], po[:], b2_sb[:], None,
                                    op0=mybir.AluOpType.add)
            om = work.tile([OUTW, R], f32, tag="om")
            nc.vector.tensor_tensor(out=om[:], in0=ob[:], in1=keep_sb[:],
                                    op=mybir.AluOpType.mult)
            nc.sync.dma_start(out[:], om[:])
    nc.compile()
    return nc


def _host_prep(preds, image, W_conv, b_conv, W1, b1, W2p, b2p, W2a, b2a, W2d, b2d):
    import ml_dtypes
    bf16 = ml_dtypes.bfloat16
    p = preds[0].astype(np.float32)
    score = p[:, 4] * p[:, 5]
    masked = np.where(score > CONF, score, -np.inf)
    idx = np.argsort(-masked, kind="stable")[:K]
    top_s = masked[idx]
    xy, wh = p[:, 0:2], p[:, 2:4]
    boxes = np.concatenate([xy - wh / 2, xy + wh / 2], axis=-1)
    b = boxes[idx]
    valid = top_s > CONF
    x1, y1, x2, y2 = b[:, 0], b[:, 1], b[:, 2], b[:, 3]
    area = (x2 - x1) * (y2 - y1)
    iw = np.clip(np.minimum(x2[:, None], x2[None, :]) - np.maximum(x1[:, None], x1[None, :]), 0, None)
    ih = np.clip(np.minimum(y2[:, None], y2[None, :]) - np.maximum(y1[:, None], y1[None, :]), 0, None)
    iou = iw * ih / (area[:, None] + area[None, :] - iw * ih + 1e-7)
    keep = valid.copy()
    for i in range(K):
        sup = np.any((iou[i, :i] > IOU) & keep[:i])
        keep[i] = keep[i] & ~sup

    xi = np.clip(np.round(b[:, 0]).astype(np.int32), 0, IMG - INP)
    yi = np.clip(np.round(b[:, 1]).astype(np.int32), 0, IMG - INP)
    img0 = image[0]
    pad = np.zeros((R, 3, 66, 66), np.float32)
    for r in range(K):
        pad[r, :, 1:65, 1:65] = img0[:, yi[r]:yi[r] + 64, xi[r]:xi[r] + 64]

    from numpy.lib.stride_tricks import sliding_window_view
    # patches[roi, ic, y, x, ky, kx]
    patches = sliding_window_view(pad, (3, 3), axis=(2, 3))
    cols_all = np.empty((8, 108, NT * 1024), bf16)
    for c in range(8):
        sub = patches[:, :, 8 * c:8 * c + 8]            # [R,3,8,64,3,3]
        arr = sub.reshape(R, 3, 4, 2, 2, 16, 2, 3, 3)   # roi,ic,ph,ydup,pxh,px,par,ky,kx
        cols_all[c] = arr.transpose(2, 1, 7, 8, 0, 3, 6, 4, 5).reshape(108, NT * 1024)

    wstk = np.zeros((108, 64), np.float32)
    wc = W_conv.reshape(16, 27).T  # [27, 16]
    for ph in range(4):
        wstk[ph * 27:(ph + 1) * 27, ph * 16:(ph + 1) * 16] = wc
    bc128 = np.tile(b_conv.astype(np.float32), 8).reshape(128, 1)

    # w1s[core][k=(pxh,ph,ch), h, q=px%16, d]
    W1r = W1.reshape(8, 16, 32, 32, 128)  # [h, ch, py, px, d]
    w1s_all = np.empty((8, 128, 8, 16, 128), bf16)
    for c in range(8):
        blk = W1r[:, :, 4 * c:4 * c + 4, :, :]          # [h, ch, ph, px32, d]
        t = blk.reshape(8, 16, 4, 2, 16, 128)           # [h,ch,ph,pxh,px,d]
        w1s_all[c] = t.transpose(3, 2, 1, 0, 4, 5).reshape(128, 8, 16, 128)

    w2_all = np.zeros((8, 128, OUTW), np.float32)
    b2_all = np.zeros((8, OUTW, 1), np.float32)
    w2_all[0, :, :PROV] = W2p; b2_all[0, :PROV, 0] = b2p
    w2_all[1, :, :ALPHA] = W2a; b2_all[1, :ALPHA, 0] = b2a
    for j in range(6):
        w2_all[2 + j, :, :AD] = W2d[j]; b2_all[2 + j, :AD, 0] = b2d[j]

    keepf = np.zeros((R,), np.float32)
    keepf[:K] = keep.astype(np.float32)
    keepm = np.broadcast_to(keepf, (OUTW, R)).copy()

    in_maps = []
    for c in range(8):
        in_maps.append({
            "cols": cols_all[c],
            "wstk": wstk.astype(bf16),
            "bc128": bc128,
            "w1s": w1s_all[c],
            "b1c": b1[c].reshape(128, 1).astype(np.float32),
            "w2": w2_all[c].astype(bf16),
            "b2": b2_all[c],
            "keepm": keepm,
        })
    return in_maps


def kernel(**inputs):
    from concourse import bass_utils
    if "nc" not in _CACHE:
        _CACHE["nc"] = _build_bass()
    nc = _CACHE["nc"]
    in_maps = _host_prep(**{k: np.asarray(v) for k, v in inputs.items()})
    res = bass_utils.run_bass_kernel_spmd(nc, in_maps, core_ids=list(range(N_CORES)))
    _CACHE["last_res"] = res
    outs = [res.results[c]["out"].T for c in range(N_CORES)]  # [304, 40] each
    logits = np.concatenate(
        [outs[0][:K, :PROV], outs[1][:K, :ALPHA]]
        + [outs[2 + j][:K, :AD] for j in range(6)], axis=1)
    return logits.astype(np.float32)
